# revision 1
# baseline (speedup 1.0000x reference)
"""Two-layer GAT on 8 Trainium2 NeuronCores (Bass/Tile).

Strategy (graph-parallel over destination nodes):
  * Host: add self-loops, sort edges by dst, split the 50000 dst nodes into
    8 shards of 6250 (one per core), chunk each shard into 49 chunks of 128
    dsts, pad each chunk's edge list to T tiles of 128 edges.  Attention
    vectors are folded into augmented weight matrices (index/weight prep
    only - no activation math on host).
  * Phase A (replicated): xw = x @ [W1 | W1 a1_src | W1 a1_dst] -> per-node
    table1 [50000, 264] (xw | alpha_src) and adtab [50000, 8] (alpha_dst).
  * Phase B (per core, per chunk): indirect-gather per-edge rows of table1
    by src id; build one-hot dst masks S / S_T from local-dst ids with
    iota + is_equal; ex = exp(leaky_relu(as[src] + ad[dst])) (softmax
    without max-shift - mathematically identical); scatter-matmul
    S^T @ [ex*xw | ex] accumulates numerator and denominator in PSUM;
    divide, +b1, ELU; transpose and matmul with [W2 | W2 a2_src | W2 a2_dst]
    -> per-core hw2aug shard [6250, 42].
  * AllGather hw2aug shards -> full table2 [50000, 42] on every core.
  * Phase C: same edge machinery with 41-wide rows -> out2 shard [6250, 40].
  * Host: concatenate the 8 shards.
"""
import os
import sys
import types

for _p in ("/opt/trn_rl_repo", "/root/.axon_site/_ro/trn_rl_repo"):
    if os.path.isdir(_p) and _p not in sys.path:
        sys.path.insert(0, _p)

import numpy as np


def _ensure_axon_hooks_shim():
    """bass_utils imports antenv.axon_hooks when tracing; provide a no-op
    shim if the image lacks it so tracing degrades instead of crashing."""
    try:
        import antenv
    except ImportError:
        return
    if "antenv.axon_hooks" in sys.modules:
        return
    try:
        import antenv.axon_hooks  # noqa: F401
        return
    except ImportError:
        pass
    m = types.ModuleType("antenv.axon_hooks")
    m._hook = None
    m.set_axon_ntff_profile_hook = lambda h: setattr(m, "_hook", h)
    m.get_axon_ntff_profile_hook = lambda: m._hook
    sys.modules["antenv.axon_hooks"] = m
    antenv.axon_hooks = m


_ensure_axon_hooks_shim()

import concourse.bacc as bacc
import concourse.bass as bass
import concourse.mybir as mybir
import concourse.tile as tile
from concourse import bass_utils
from concourse.masks import make_identity

# ---------------------------------------------------------------- problem dims
N = 50000
E = 800000
F_IN = 256
HEADS = 8
HID = 32
D1 = HEADS * HID          # 256
NCLS = 40
NEG = 0.2                 # leaky_relu slope
NC = 8                    # cores
NSH = N // NC             # 6250 dst nodes per core
P = 128
CHUNKS = (NSH + P - 1) // P   # 49 chunks of 128 dsts
W1COL = D1 + 2 * HEADS    # 272: xw | as | ad
T1COL = D1 + HEADS        # 264: table1 row = xw | as
W2COL = NCLS + 2          # 42: hw2 | as2 | ad2
PAD_LD = 200.0            # local-dst marker for padded edge slots

f32 = mybir.dt.float32
f32r = mybir.dt.float32r
i32 = mybir.dt.int32
AF = mybir.ActivationFunctionType
OP = mybir.AluOpType

_last_bench = None        # test.py reads timing info from here
_prog_cache = {}


# ---------------------------------------------------------------- host prep
def _prep(x, edge_index, W1, a1_src, a1_dst, b1, W2, a2_src, a2_dst, b2):
    src = np.concatenate([edge_index[0], np.arange(N, dtype=np.int32)])
    dst = np.concatenate([edge_index[1], np.arange(N, dtype=np.int32)])
    order = np.argsort(dst, kind="stable")
    src = src[order].astype(np.int32)
    dst = dst[order].astype(np.int32)

    shard_bounds = np.searchsorted(dst, np.arange(0, N + 1, NSH))
    counts = []
    for k in range(NC):
        lo, hi = shard_bounds[k], shard_bounds[k + 1]
        cb = np.searchsorted(dst[lo:hi],
                             k * NSH + np.arange(0, CHUNKS * P + 1, P))
        counts.append(np.diff(cb))
    counts = np.asarray(counts)
    T = int(np.max((counts + P - 1) // P))

    srcidx = np.zeros((NC, P, CHUNKS * T), np.int32)
    ld_a = np.full((NC, P, CHUNKS * T), PAD_LD, np.float32)
    ld_b = np.full((NC, 1, CHUNKS * T * P), PAD_LD, np.float32)
    dstid = np.zeros((NC, P, CHUNKS), np.int32)
    for k in range(NC):
        off = shard_bounds[k]
        for c in range(CHUNKS):
            m = counts[k][c]
            es = src[off:off + m]
            el = dst[off:off + m] - (k * NSH + c * P)
            off += m
            j = np.arange(m)
            p, t = j % P, j // P
            srcidx[k, p, c * T + t] = es
            ld_a[k, p, c * T + t] = el
            ld_b[k, 0, c * T * P + t * P + p] = el
        dstid[k, :, :] = np.minimum(
            k * NSH + np.arange(CHUNKS)[None, :] * P
            + np.arange(P)[:, None], N - 1)

    A1s = np.zeros((D1, HEADS), np.float32)
    A1d = np.zeros((D1, HEADS), np.float32)
    for h in range(HEADS):
        A1s[h * HID:(h + 1) * HID, h] = a1_src[h]
        A1d[h * HID:(h + 1) * HID, h] = a1_dst[h]
    W1aug = np.concatenate([W1, W1 @ A1s, W1 @ A1d], axis=1)
    W2aug = np.concatenate([W2, W2 @ a2_src.T, W2 @ a2_dst.T], axis=1)
    xT = np.ascontiguousarray(x.T)

    shared = {
        "xT": xT.astype(np.float32),
        "W1aug": W1aug.astype(np.float32),
        "W2aug": W2aug.astype(np.float32),
        "b1": b1.reshape(1, D1).astype(np.float32),
        "b2": b2.reshape(1, NCLS).astype(np.float32),
    }
    in_maps = []
    for k in range(NC):
        m = dict(shared)
        m["srcidx"] = srcidx[k]
        m["ld_a"] = ld_a[k]
        m["ld_b"] = ld_b[k]
        m["dstid"] = dstid[k]
        in_maps.append(m)
    return in_maps, T


# ---------------------------------------------------------------- program
def _build(T):
    nc = bacc.Bacc("TRN2", target_bir_lowering=False, debug=False,
                   num_devices=NC)
    g = {}
    g["xT"] = nc.dram_tensor("xT", [F_IN, N], f32r, kind="ExternalInput").ap()
    g["W1aug"] = nc.dram_tensor("W1aug", [F_IN, W1COL], f32r,
                                kind="ExternalInput").ap()
    g["W2aug"] = nc.dram_tensor("W2aug", [F_IN, W2COL], f32r,
                                kind="ExternalInput").ap()
    g["b1"] = nc.dram_tensor("b1", [1, D1], f32r, kind="ExternalInput").ap()
    g["b2"] = nc.dram_tensor("b2", [1, NCLS], f32r, kind="ExternalInput").ap()
    g["srcidx"] = nc.dram_tensor("srcidx", [P, CHUNKS * T], i32,
                                 kind="ExternalInput").ap()
    g["ld_a"] = nc.dram_tensor("ld_a", [P, CHUNKS * T], f32,
                               kind="ExternalInput").ap()
    g["ld_b"] = nc.dram_tensor("ld_b", [1, CHUNKS * T * P], f32r,
                               kind="ExternalInput").ap()
    g["dstid"] = nc.dram_tensor("dstid", [P, CHUNKS], i32,
                                kind="ExternalInput").ap()
    g["out2"] = nc.dram_tensor("out2", [NSH, NCLS], f32,
                               kind="ExternalOutput").ap()
    g["table1"] = nc.dram_tensor("table1", [N, T1COL], f32r,
                                 kind="Internal").ap()
    g["adtab"] = nc.dram_tensor("adtab", [N, HEADS], f32,
                                kind="Internal").ap()

    with tile.TileContext(nc) as tc:
        _emit(nc, tc, T, g)
    nc.compile()
    return nc


def _emit(nc, tc, T, g):
    # ---------------- resident tiles
    with tc.tile_pool(name="res", bufs=1) as res:
        w1_sb = res.tile([P, 2, W1COL], f32r)
        nc.sync.dma_start(out=w1_sb[:],
                          in_=g["W1aug"][:, :].rearrange("(h p) c -> p h c",
                                                         h=2))
        w2_sb = res.tile([P, 2, W2COL], f32r)
        nc.sync.dma_start(out=w2_sb[:],
                          in_=g["W2aug"][:, :].rearrange("(h p) c -> p h c",
                                                         h=2))
        srcidx_sb = res.tile([P, CHUNKS * T], i32)
        nc.sync.dma_start(out=srcidx_sb[:], in_=g["srcidx"][:, :])
        ld_a_sb = res.tile([P, CHUNKS * T], f32)
        nc.sync.dma_start(out=ld_a_sb[:], in_=g["ld_a"][:, :])
        dstid_sb = res.tile([P, CHUNKS], i32)
        nc.sync.dma_start(out=dstid_sb[:], in_=g["dstid"][:, :])

        iota_i = res.tile([P, P], i32)
        nc.gpsimd.iota(iota_i[:], pattern=[[1, P]], base=0,
                       channel_multiplier=0)
        iota_f = res.tile([P, P], f32)
        nc.vector.tensor_copy(out=iota_f[:], in_=iota_i[:])
        iotac_i = res.tile([P, 1], i32)
        nc.gpsimd.iota(iotac_i[:], pattern=[[0, 1]], base=0,
                       channel_multiplier=1)
        iotac_f = res.tile([P, 1], f32)
        nc.vector.tensor_copy(out=iotac_f[:], in_=iotac_i[:])
        ones_f = res.tile([1, P], f32)
        nc.vector.memset(ones_f[:], 1.0)
        ones_sb = res.tile([1, P], f32r)
        nc.vector.tensor_copy(out=ones_sb[:], in_=ones_f[:])
        ident_f = res.tile([P, P], f32)
        make_identity(nc, ident_f[:])
        ident = res.tile([P, P], f32r)
        nc.vector.tensor_copy(out=ident[:], in_=ident_f[:])

        # partition-broadcast biases via ones-matmul
        with tc.tile_pool(name="bb", bufs=1, space="PSUM") as bbp:
            b1_row = res.tile([1, D1], f32r)
            nc.sync.dma_start(out=b1_row[:], in_=g["b1"][:, :])
            b2_row = res.tile([1, NCLS], f32r)
            nc.sync.dma_start(out=b2_row[:], in_=g["b2"][:, :])
            b1_ps = bbp.tile([P, D1], f32, space="PSUM", tag="b")
            nc.tensor.matmul(out=b1_ps[:], lhsT=ones_sb[:], rhs=b1_row[:],
                             start=True, stop=True)
            b1_bc = res.tile([P, D1], f32)
            nc.vector.tensor_copy(out=b1_bc[:], in_=b1_ps[:])
            b2_ps = bbp.tile([P, NCLS], f32, space="PSUM", tag="b")
            nc.tensor.matmul(out=b2_ps[:], lhsT=ones_sb[:], rhs=b2_row[:],
                             start=True, stop=True)
            b2_bc = res.tile([P, NCLS], f32)
            nc.vector.tensor_copy(out=b2_bc[:], in_=b2_ps[:])

        # ---------------- Phase A: table1/adtab for all nodes (replicated)
        SUP = 8
        NBLK = (N + P - 1) // P
        with tc.tile_pool(name="pa_x", bufs=2) as pax, \
             tc.tile_pool(name="pa_o", bufs=2) as pao, \
             tc.tile_pool(name="pa_ps", bufs=2, space="PSUM") as paps:
            for s0 in range(0, NBLK, SUP):
                nsub = min(SUP, NBLK - s0)
                ncols = min(N - s0 * P, nsub * P)
                xt_t = pax.tile([P, 2, SUP * P], f32r, tag="x")
                nc.sync.dma_start(
                    out=xt_t[:, :, :ncols],
                    in_=g["xT"][:, :].rearrange("(h p) n -> p h n", h=2)
                    [:, :, s0 * P:s0 * P + ncols])
                o_t = pao.tile([P, SUP, W1COL], f32, tag="o")
                for j in range(nsub):
                    cw = min(P, ncols - j * P)
                    ps = paps.tile([P, W1COL], f32, space="PSUM", tag="ps")
                    for h in range(2):
                        nc.tensor.matmul(
                            out=ps[:cw, :],
                            lhsT=xt_t[:, h, j * P:j * P + cw],
                            rhs=w1_sb[:, h, :],
                            start=(h == 0), stop=(h == 1))
                    nc.vector.tensor_copy(out=o_t[:cw, j, :], in_=ps[:cw, :])
                    r0 = (s0 + j) * P
                    nc.sync.dma_start(
                        out=g["table1"][r0:r0 + cw, :],
                        in_=o_t[:cw, j, :T1COL].bitcast(f32r))
                    nc.sync.dma_start(
                        out=g["adtab"][r0:r0 + cw, :],
                        in_=o_t[:cw, j, T1COL:])

        # ---------------- a_d rows for this core's dst shard
        adv_sb = res.tile([P, CHUNKS * HEADS], f32r)
        with tc.tile_pool(name="adp", bufs=2) as adp_pool:
            for c in range(CHUNKS):
                t_ad = adp_pool.tile([P, HEADS], f32r, tag="ad")
                nc.gpsimd.indirect_dma_start(
                    out=t_ad[:], out_offset=None, in_=g["adtab"][:, :],
                    in_offset=bass.IndirectOffsetOnAxis(
                        ap=dstid_sb[:, c:c + 1], axis=0))
                nc.vector.tensor_copy(
                    out=adv_sb[:, c * HEADS:(c + 1) * HEADS], in_=t_ad[:])

        # ---------------- Phase B, AllGather, Phase C
        with tc.tile_pool(name="dram", bufs=1, space="DRAM") as dr:
            hw2_shard = dr.tile([NSH, W2COL], f32)
            table2 = dr.tile([N, W2COL], f32)

            _edge_layer(
                nc, tc, T, layer=1, table=g["table1"], row_w=T1COL,
                nheads=HEADS, hid=HID, adv_sb=adv_sb, srcidx_sb=srcidx_sb,
                ld_a_sb=ld_a_sb, ld_b=g["ld_b"], iota_f=iota_f,
                iotac_f=iotac_f, ones_sb=ones_sb, ident=ident, b_bc=b1_bc,
                w2_sb=w2_sb, hw2_shard=hw2_shard, out2=None)

            nc.gpsimd.collective_compute(
                "AllGather", OP.bypass,
                replica_groups=[list(range(NC))],
                ins=[hw2_shard[:].opt()], outs=[table2[:].opt()])

            adv2_sb = res.tile([P, 2 * CHUNKS], f32r)
            with tc.tile_pool(name="ad2", bufs=2) as ad2_pool:
                for c in range(CHUNKS):
                    t_r = ad2_pool.tile([P, W2COL], f32r, tag="r")
                    nc.gpsimd.indirect_dma_start(
                        out=t_r[:], out_offset=None, in_=table2[:],
                        in_offset=bass.IndirectOffsetOnAxis(
                            ap=dstid_sb[:, c:c + 1], axis=0))
                    nc.vector.tensor_copy(
                        out=adv2_sb[:, 2 * c:2 * c + 2],
                        in_=t_r[:, W2COL - 1:W2COL].to_broadcast((P, 2)))

            _edge_layer(
                nc, tc, T, layer=2, table=table2[:].bitcast(f32r),
                row_w=W2COL, nheads=1, hid=NCLS, adv_sb=adv2_sb,
                srcidx_sb=srcidx_sb, ld_a_sb=ld_a_sb, ld_b=g["ld_b"],
                iota_f=iota_f, iotac_f=iotac_f, ones_sb=ones_sb, ident=ident,
                b_bc=b2_bc, w2_sb=None, hw2_shard=None, out2=g["out2"])


def _edge_layer(nc, tc, T, layer, table, row_w, nheads, hid, adv_sb,
                srcidx_sb, ld_a_sb, ld_b, iota_f, iotac_f, ones_sb, ident,
                b_bc, w2_sb, hw2_shard, out2):
    """One GAT message-passing layer over this core's 49 dst chunks."""
    NH = nheads
    DW = NH * hid                    # payload width (256 or 40)
    GW = row_w                       # gathered row width (264 or 42)
    EW = T * P                       # edges per chunk (padded)
    sfx = f"l{layer}"

    with tc.tile_pool(name=f"g{sfx}", bufs=2) as gpool, \
         tc.tile_pool(name=f"s{sfx}", bufs=2) as spool, \
         tc.tile_pool(name=f"m{sfx}", bufs=2) as mpool, \
         tc.tile_pool(name=f"lb{sfx}", bufs=1, space="PSUM") as lbp, \
         tc.tile_pool(name=f"ac{sfx}", bufs=1, space="PSUM") as acp, \
         tc.tile_pool(name=f"sm{sfx}", bufs=1, space="PSUM") as smp:
        for c in range(CHUNKS):
            # ---- gather per-edge rows (one indirect DMA per 128-edge tile)
            G = gpool.tile([P, T, GW], f32r, tag="g")
            for t in range(T):
                nc.gpsimd.indirect_dma_start(
                    out=G[:, t, :], out_offset=None, in_=table[:, :],
                    in_offset=bass.IndirectOffsetOnAxis(
                        ap=srcidx_sb[:, c * T + t:c * T + t + 1], axis=0))

            # ---- masks
            S = spool.tile([P, T, P], f32r, tag="S")
            nc.vector.tensor_tensor(
                out=S[:],
                in0=iota_f[:].unsqueeze(1).to_broadcast((P, T, P)),
                in1=ld_a_sb[:, c * T:(c + 1) * T].unsqueeze(2)
                .to_broadcast((P, T, P)),
                op=OP.is_equal)

            ldrow = mpool.tile([1, EW], f32r, tag="ldr")
            nc.sync.dma_start(out=ldrow[:],
                              in_=ld_b[:, c * EW:(c + 1) * EW])
            S_T = spool.tile([P, EW], f32r, tag="ST")
            HCOL = ((T + 1) // 2) * P       # half width, tile-aligned
            for hh in range(2):
                h0 = hh * HCOL
                hw_ = min(EW - h0, HCOL)
                if hw_ <= 0:
                    continue
                ldb_ps = lbp.tile([P, HCOL], f32, space="PSUM", tag="ldb")
                for q0 in range(0, hw_, 512):
                    qw = min(512, hw_ - q0)
                    nc.tensor.matmul(out=ldb_ps[:, q0:q0 + qw],
                                     lhsT=ones_sb[:],
                                     rhs=ldrow[:, h0 + q0:h0 + q0 + qw],
                                     start=True, stop=True)
                nc.vector.tensor_tensor(
                    out=S_T[:, h0:h0 + hw_], in0=ldb_ps[:, :hw_],
                    in1=iotac_f[:].to_broadcast((P, hw_)),
                    op=OP.is_equal)

            # ---- ad per edge: S_T_t^T @ ad_chunk (APAD: fp32r even-dim)
            APAD = NH if NH % 2 == 0 else NH + 1
            adpe_ps = smp.tile([P, T * APAD], f32, space="PSUM", tag="adpe")
            for t in range(T):
                nc.tensor.matmul(
                    out=adpe_ps[:, t * APAD:(t + 1) * APAD],
                    lhsT=S_T[:, t * P:(t + 1) * P],
                    rhs=adv_sb[:, c * APAD:(c + 1) * APAD],
                    start=True, stop=True)

            # ---- ex = exp(lrelu(as + ad)) written into G[:, :, DW:DW+NH]
            logit = mpool.tile([P, T * NH], f32, tag="logit")
            nc.vector.tensor_tensor(
                out=logit[:].rearrange("p (t h) -> p t h", t=T),
                in0=G[:, :, DW:DW + NH],
                in1=adpe_ps[:].rearrange("p (t a) -> p t a", t=T)[:, :, :NH],
                op=OP.add)
            lr_p = mpool.tile([P, T * NH], f32, tag="lrp")
            nc.vector.tensor_scalar(out=lr_p[:], in0=logit[:], scalar1=0.0,
                                    scalar2=None, op0=OP.max)
            lr_n = mpool.tile([P, T * NH], f32, tag="lrn")
            nc.vector.tensor_scalar(out=lr_n[:], in0=logit[:], scalar1=0.0,
                                    scalar2=NEG, op0=OP.min, op1=OP.mult)
            nc.vector.tensor_tensor(out=logit[:], in0=lr_p[:], in1=lr_n[:],
                                    op=OP.add)
            nc.scalar.activation(
                out=G[:, :, DW:DW + NH],
                in_=logit[:].rearrange("p (t h) -> p t h", t=T),
                func=AF.Exp)

            # ---- weight features by ex (in place)
            nc.vector.tensor_tensor(
                out=G[:, :, :DW].rearrange("p t (h w) -> p t h w", h=NH),
                in0=G[:, :, :DW].rearrange("p t (h w) -> p t h w", h=NH),
                in1=G[:, :, DW:DW + NH].unsqueeze(3)
                .to_broadcast((P, T, NH, hid)),
                op=OP.mult)

            # ---- scatter: acc[d, :] = sum_t S_t^T @ G_t
            # (fp32r matmul needs even free dims; GW=42 covers the unused
            # ad2 column in layer 2)
            accw = DW + NH if (DW + NH) % 2 == 0 else GW
            acc = acp.tile([P, accw], f32, space="PSUM", tag="acc")
            for t in range(T):
                nc.tensor.matmul(out=acc[:], lhsT=S[:, t, :],
                                 rhs=G[:, t, :accw],
                                 start=(t == 0), stop=(t == T - 1))

            # ---- epilogue: divide by denom, bias
            den = mpool.tile([P, NH], f32, tag="den")
            nc.vector.tensor_scalar(out=den[:], in0=acc[:, DW:DW + NH],
                                    scalar1=1e-30, scalar2=None, op0=OP.max)
            rec = mpool.tile([P, NH], f32, tag="rec")
            nc.vector.reciprocal(out=rec[:], in_=den[:])
            outv = mpool.tile([P, DW], f32, tag="outv")
            nc.vector.tensor_tensor(
                out=outv[:].rearrange("p (h w) -> p h w", h=NH),
                in0=acc[:, :DW].rearrange("p (h w) -> p h w", h=NH),
                in1=rec[:].unsqueeze(2).to_broadcast((P, NH, hid)),
                op=OP.mult)
            nc.vector.tensor_tensor(out=outv[:], in0=outv[:], in1=b_bc[:],
                                    op=OP.add)

            rows = min(P, NSH - c * P)
            if layer == 2:
                nc.sync.dma_start(out=out2[c * P:c * P + rows, :],
                                  in_=outv[:rows, :])
                continue

            # ---- layer 1: ELU, then h @ W2aug -> hw2_shard rows
            mneg = mpool.tile([P, DW], f32, tag="mneg")
            nc.vector.tensor_scalar(out=mneg[:], in0=outv[:], scalar1=0.0,
                                    scalar2=None, op0=OP.min)
            expm = mpool.tile([P, DW], f32, tag="expm")
            nc.scalar.activation(out=expm[:], in_=mneg[:], func=AF.Exp)
            rel1 = mpool.tile([P, DW], f32, tag="rel1")
            nc.vector.tensor_scalar(out=rel1[:], in0=outv[:], scalar1=0.0,
                                    scalar2=1.0, op0=OP.max, op1=OP.subtract)
            h_sb = mpool.tile([P, DW], f32r, tag="h")
            nc.vector.tensor_tensor(out=h_sb[:], in0=expm[:], in1=rel1[:],
                                    op=OP.add)

            hT_ps = smp.tile([P, P], f32r, space="PSUM", tag="hT")
            hT_sb = mpool.tile([P, 2, P], f32r, tag="hTs")
            for hhalf in range(2):
                nc.tensor.transpose(out=hT_ps[:],
                                    in_=h_sb[:, hhalf * P:(hhalf + 1) * P],
                                    identity=ident[:])
                nc.vector.tensor_copy(out=hT_sb[:, hhalf, :], in_=hT_ps[:])
            hw_ps = smp.tile([P, W2COL], f32, space="PSUM", tag="hw")
            for hhalf in range(2):
                nc.tensor.matmul(out=hw_ps[:], lhsT=hT_sb[:, hhalf, :],
                                 rhs=w2_sb[:, hhalf, :],
                                 start=(hhalf == 0), stop=(hhalf == 1))
            hw_sb = mpool.tile([P, W2COL], f32, tag="hws")
            nc.vector.tensor_copy(out=hw_sb[:], in_=hw_ps[:])
            nc.sync.dma_start(out=hw2_shard[c * P:c * P + rows, :],
                              in_=hw_sb[:rows, :])


# ---------------------------------------------------------------- entry
def kernel(**inputs):
    global _last_bench
    args = {k: np.asarray(v) for k, v in inputs.items()}
    in_maps, T = _prep(
        args["x"], args["edge_index"], args["W1"], args["a1_src"],
        args["a1_dst"], args["b1"], args["W2"], args["a2_src"],
        args["a2_dst"], args["b2"])
    if T not in _prog_cache:
        _prog_cache[T] = _build(T)
    nc = _prog_cache[T]
    trace = os.environ.get("GAT_TRACE", "0") == "1"
    r = bass_utils.run_bass_kernel_spmd(
        nc, in_maps, core_ids=list(range(NC)), trace=trace)
    _last_bench = r
    out = np.concatenate([r.results[k]["out2"] for k in range(NC)], axis=0)
    return out.astype(np.float32)



# revision 2
# speedup vs baseline: 1.0247x; 1.0247x over previous
"""Two-layer GAT on 8 Trainium2 NeuronCores (Bass/Tile) — v3.

Destination-major edge layout: edge slot (p = dst rank within chunk,
t = in-edge index). Chunks are 128 dsts of similar degree (degree-sorted
permutation per core), so the per-chunk tile count J[c] tracks the chunk's
max degree with little padding.

  * No one-hot masks: the scatter is J[c] identity-lhsT matmuls that
    accumulate [128, SCW] in PSUM (a per-partition reduce over t).
  * alpha_dst is per-partition: one narrow indirect gather per chunk (L1),
    a direct column read (L2); broadcast along t at 2x.
  * Pad slots point at a sentinel table row (alpha_src = -80 -> ex ~ 1e-7,
    zero payload) so they vanish from numerator and denominator.
  * Gathers are per-tile [P, 1]-offset indirect DMAs (the only shape the
    SWDGE ucode supports).
bf16 data path; sharded Phase A + AllGather of the projected tables.
"""
import os
import sys
import types

for _p in ("/opt/trn_rl_repo", "/root/.axon_site/_ro/trn_rl_repo"):
    if os.path.isdir(_p) and _p not in sys.path:
        sys.path.insert(0, _p)

import numpy as np
import ml_dtypes


def _ensure_axon_hooks_shim():
    try:
        import antenv
    except ImportError:
        return
    if "antenv.axon_hooks" in sys.modules:
        return
    try:
        import antenv.axon_hooks  # noqa: F401
        return
    except ImportError:
        pass
    m = types.ModuleType("antenv.axon_hooks")
    m._hook = None
    m.set_axon_ntff_profile_hook = lambda h: setattr(m, "_hook", h)
    m.get_axon_ntff_profile_hook = lambda: m._hook
    sys.modules["antenv.axon_hooks"] = m
    antenv.axon_hooks = m


_ensure_axon_hooks_shim()

import concourse.bacc as bacc
import concourse.bass as bass
import concourse.mybir as mybir
import concourse.tile as tile
from concourse import bass_utils
from concourse.masks import make_identity

# ---------------------------------------------------------------- problem dims
N = 50000
E = 800000
F_IN = 256
HEADS = 8
HID = 32
D1 = HEADS * HID          # 256
NCLS = 40
NEG = 0.2
NC = 8
NSH = N // NC             # dst nodes per core
P = 128
T1COL = D1 + HEADS        # 264 = xw | alpha_src
W1COL = D1 + 2 * HEADS    # 272 = xw | as | ad
W2COL = 44                # hw2(40) | as2 | ad2 | one | pad
SENT_AS = -80.0
EPS = 1e-20

f32 = mybir.dt.float32
bf16 = mybir.dt.bfloat16
i32 = mybir.dt.int32
AF = mybir.ActivationFunctionType
OP = mybir.AluOpType

_last_bench = None
_prog_cache = {}

bfloat16 = ml_dtypes.bfloat16


def _nchunk():
    return (NSH + P - 1) // P


# ---------------------------------------------------------------- host prep
def _prep(x, edge_index, W1, a1_src, a1_dst, b1, W2, a2_src, a2_dst, b2):
    NCH = _nchunk()
    SLOT1 = NSH + 1
    SLOT2 = NCH * P + 1
    src = np.concatenate([edge_index[0], np.arange(N, dtype=np.int32)])
    dst = np.concatenate([edge_index[1], np.arange(N, dtype=np.int32)])
    order = np.argsort(dst, kind="stable")
    src = src[order].astype(np.int64)
    dst = dst[order].astype(np.int64)
    bounds = np.searchsorted(dst, np.arange(0, N + 1, NSH))

    perms = np.empty((NC, NSH), np.int64)
    rankof = np.empty((NC, NSH), np.int64)
    degs = np.empty((NC, NSH), np.int64)
    for k in range(NC):
        lo, hi = bounds[k], bounds[k + 1]
        deg = np.bincount(dst[lo:hi] - k * NSH, minlength=NSH)
        perm = np.argsort(-deg, kind="stable")
        perms[k] = perm
        rankof[k][perm] = np.arange(NSH)
        degs[k] = deg

    J = np.ones(NCH, np.int64)
    for k in range(NC):
        dsort = degs[k][perms[k]]
        for c in range(NCH):
            seg = dsort[c * P:(c + 1) * P]
            if len(seg):
                J[c] = max(J[c], int(seg.max()))
    TCOLS = int(J.sum())
    colbase = np.concatenate([[0], np.cumsum(J)]).astype(np.int64)

    srcidx1 = np.full((NC, P, TCOLS), NSH, np.int32)        # pad -> sentinel
    srcidx2 = np.full((NC, P, TCOLS), NCH * P, np.int32)
    adidx = np.zeros((NC, P, NCH), np.int32)

    for k in range(NC):
        lo, hi = bounds[k], bounds[k + 1]
        dl = dst[lo:hi] - k * NSH
        ss = src[lo:hi]
        rank = rankof[k][dl]
        c = rank // P
        p = rank % P
        idxs = np.arange(hi - lo)
        runstart = np.r_[True, dl[1:] != dl[:-1]]
        startpos = np.maximum.accumulate(np.where(runstart, idxs, 0))
        t = idxs - startpos
        assert (t < J[c]).all()
        col = colbase[c] + t
        sk = ss // NSH
        sl = ss % NSH
        srcidx1[k, p, col] = sk * SLOT1 + sl
        srcidx2[k, p, col] = sk * SLOT2 + rankof[sk, sl]
        grid = np.minimum(np.arange(NCH)[None, :] * P
                          + np.arange(P)[:, None], NSH - 1)
        adidx[k] = perms[k][grid]

    A1s = np.zeros((D1, HEADS), np.float32)
    A1d = np.zeros((D1, HEADS), np.float32)
    for h in range(HEADS):
        A1s[h * HID:(h + 1) * HID, h] = a1_src[h]
        A1d[h * HID:(h + 1) * HID, h] = a1_dst[h]
    W1aug = np.concatenate([W1, W1 @ A1s, W1 @ A1d], axis=1)
    W2aug = np.zeros((D1, W2COL), np.float32)
    W2aug[:, :NCLS] = W2
    W2aug[:, NCLS] = (W2 @ a2_src.T)[:, 0]
    W2aug[:, NCLS + 1] = (W2 @ a2_dst.T)[:, 0]
    onecol = np.zeros((1, W2COL), np.float32)
    onecol[0, NCLS + 2] = 1.0
    sent1 = np.zeros((1, T1COL), np.float32)
    sent1[0, D1:] = SENT_AS
    sent2 = np.zeros((1, W2COL), np.float32)
    sent2[0, NCLS] = SENT_AS

    xT = np.ascontiguousarray(x.T).astype(bfloat16)

    shared = {
        "W1aug": W1aug.astype(bfloat16),
        "W2aug": W2aug.astype(bfloat16),
        "b1": b1.reshape(1, D1).astype(bfloat16),
        "onecol": onecol.astype(bfloat16),
        "b2": b2.reshape(1, NCLS).astype(bfloat16),
        "sent1": sent1.astype(bfloat16),
        "sent2": sent2.astype(bfloat16),
    }
    in_maps = []
    for k in range(NC):
        mdl = dict(shared)
        mdl["xTs"] = np.ascontiguousarray(xT[:, k * NSH:(k + 1) * NSH])
        mdl["srcidx1"] = srcidx1[k]
        mdl["srcidx2"] = srcidx2[k]
        mdl["adidx"] = adidx[k]
        in_maps.append(mdl)
    return in_maps, tuple(int(j) for j in J), perms


# ---------------------------------------------------------------- program
def _build(J):
    NCH = _nchunk()
    SLOT1 = NSH + 1
    SLOT2 = NCH * P + 1
    TCOLS = int(sum(J))
    nc = bacc.Bacc("TRN2", target_bir_lowering=False, debug=False,
                   num_devices=NC)
    g = {}

    def ein(name, shape, dt):
        g[name] = nc.dram_tensor(name, shape, dt, kind="ExternalInput").ap()

    ein("xTs", [F_IN, NSH], bf16)
    ein("W1aug", [F_IN, W1COL], bf16)
    ein("W2aug", [D1, W2COL], bf16)
    ein("b1", [1, D1], bf16)
    ein("onecol", [1, W2COL], bf16)
    ein("b2", [1, NCLS], bf16)
    ein("sent1", [1, T1COL], bf16)
    ein("sent2", [1, W2COL], bf16)
    ein("srcidx1", [P, TCOLS], i32)
    ein("srcidx2", [P, TCOLS], i32)
    ein("adidx", [P, NCH], i32)
    g["out2"] = nc.dram_tensor("out2", [NCH * P, NCLS], f32,
                               kind="ExternalOutput").ap()
    g["t1shard"] = nc.dram_tensor("t1shard", [SLOT1, T1COL], bf16,
                                  kind="Internal").ap()
    g["adshard"] = nc.dram_tensor("adshard", [NSH, HEADS], bf16,
                                  kind="Internal").ap()
    g["table1"] = nc.dram_tensor("table1", [NC * SLOT1, T1COL], bf16,
                                 kind="Internal", addr_space="Shared").ap()
    g["hw2slots"] = nc.dram_tensor("hw2slots", [SLOT2, W2COL], bf16,
                                   kind="Internal").ap()
    g["table2"] = nc.dram_tensor("table2", [NC * SLOT2, W2COL], bf16,
                                 kind="Internal", addr_space="Shared").ap()

    with tile.TileContext(nc) as tc:
        _emit(nc, tc, J, g)
    nc.compile()
    return nc


def _emit(nc, tc, J, g):
    NCH = _nchunk()
    TCOLS = int(sum(J))
    with tc.tile_pool(name="res", bufs=1) as res:
        w1_sb = res.tile([P, 2, W1COL], bf16)
        nc.sync.dma_start(out=w1_sb[:],
                          in_=g["W1aug"][:, :].rearrange("(h p) c -> p h c",
                                                         h=2))
        w2_sb = res.tile([P, 2, W2COL], bf16)
        nc.sync.dma_start(out=w2_sb[:],
                          in_=g["W2aug"][:, :].rearrange("(h p) c -> p h c",
                                                         h=2))
        srcidx1_sb = res.tile([P, TCOLS], i32)
        nc.sync.dma_start(out=srcidx1_sb[:], in_=g["srcidx1"][:, :])
        srcidx2_sb = res.tile([P, TCOLS], i32)
        nc.sync.dma_start(out=srcidx2_sb[:], in_=g["srcidx2"][:, :])
        adidx_sb = res.tile([P, NCH], i32)
        nc.sync.dma_start(out=adidx_sb[:], in_=g["adidx"][:, :])

        ident_f = res.tile([P, P], f32)
        make_identity(nc, ident_f[:])
        ident = res.tile([P, P], bf16)
        nc.vector.tensor_copy(out=ident[:], in_=ident_f[:])
        ones_sb = res.tile([1, P], bf16)
        nc.vector.memset(ones_sb[:], 1.0)

        with tc.tile_pool(name="bb", bufs=1, space="PSUM") as bbp:
            b1_row = res.tile([1, D1], bf16)
            nc.sync.dma_start(out=b1_row[:], in_=g["b1"][:, :])
            b2_row = res.tile([1, NCLS], bf16)
            nc.sync.dma_start(out=b2_row[:], in_=g["b2"][:, :])
            one_row = res.tile([1, W2COL], bf16)
            nc.sync.dma_start(out=one_row[:], in_=g["onecol"][:, :])
            b1_ps = bbp.tile([P, D1], f32, space="PSUM", tag="b")
            nc.tensor.matmul(out=b1_ps[:], lhsT=ones_sb[:], rhs=b1_row[:],
                             start=True, stop=True)
            b1_bc = res.tile([P, D1], f32)
            nc.vector.tensor_copy(out=b1_bc[:], in_=b1_ps[:])
            b2_ps = bbp.tile([P, NCLS], f32, space="PSUM", tag="b")
            nc.tensor.matmul(out=b2_ps[:], lhsT=ones_sb[:], rhs=b2_row[:],
                             start=True, stop=True)
            b2_bc = res.tile([P, NCLS], f32)
            nc.vector.tensor_copy(out=b2_bc[:], in_=b2_ps[:])
            one_ps = bbp.tile([P, W2COL], f32, space="PSUM", tag="b")
            nc.tensor.matmul(out=one_ps[:], lhsT=ones_sb[:], rhs=one_row[:],
                             start=True, stop=True)
            one_bc = res.tile([P, W2COL], f32)
            nc.vector.tensor_copy(out=one_bc[:], in_=one_ps[:])

        # ---------------- Phase A (own shard)
        NBLK = (NSH + P - 1) // P
        with tc.tile_pool(name="pa_x", bufs=1) as pax, \
             tc.tile_pool(name="pa_o", bufs=3) as pao, \
             tc.tile_pool(name="pa_ps", bufs=2, space="PSUM") as paps:
            xts = pax.tile([P, 2, NSH], bf16)
            nc.sync.dma_start(
                out=xts[:],
                in_=g["xTs"][:, :].rearrange("(h p) n -> p h n", h=2))
            for j in range(NBLK):
                cw = min(P, NSH - j * P)
                ps = paps.tile([P, W1COL], f32, space="PSUM", tag="ps")
                for h in range(2):
                    nc.tensor.matmul(out=ps[:cw, :],
                                     lhsT=xts[:, h, j * P:j * P + cw],
                                     rhs=w1_sb[:, h, :],
                                     start=(h == 0), stop=(h == 1))
                o_t = pao.tile([P, W1COL], bf16, tag="o")
                if j % 2 == 0:
                    nc.scalar.copy(out=o_t[:cw, :], in_=ps[:cw, :])
                else:
                    nc.vector.tensor_copy(out=o_t[:cw, :], in_=ps[:cw, :])
                nc.sync.dma_start(out=g["t1shard"][j * P:j * P + cw, :],
                                  in_=o_t[:cw, :T1COL])
                nc.sync.dma_start(out=g["adshard"][j * P:j * P + cw, :],
                                  in_=o_t[:cw, T1COL:])
            sent_sb = pax.tile([1, T1COL], bf16)
            nc.sync.dma_start(out=sent_sb[:], in_=g["sent1"][:, :])
            nc.sync.dma_start(out=g["t1shard"][NSH:NSH + 1, :],
                              in_=sent_sb[:])

        nc.gpsimd.collective_compute(
            "AllGather", OP.bypass,
            replica_groups=[list(range(NC))],
            ins=[g["t1shard"][:, :].opt()], outs=[g["table1"][:, :].opt()])

        _edge_layer(nc, tc, J, layer=1, table=g["table1"], row_w=T1COL,
                    nheads=HEADS, hid=HID, srcidx_sb=srcidx1_sb,
                    adidx_sb=adidx_sb, adshard=g["adshard"], b_bc=b1_bc,
                    one_bc=one_bc, ident=ident, w2_sb=w2_sb,
                    hw2slots=g["hw2slots"], out2=None, g=g)

        nc.gpsimd.collective_compute(
            "AllGather", OP.bypass,
            replica_groups=[list(range(NC))],
            ins=[g["hw2slots"][:, :].opt()], outs=[g["table2"][:, :].opt()])

        _edge_layer(nc, tc, J, layer=2, table=g["table2"], row_w=W2COL,
                    nheads=1, hid=NCLS, srcidx_sb=srcidx2_sb,
                    adidx_sb=None, adshard=g["hw2slots"], b_bc=b2_bc,
                    one_bc=None, ident=ident, w2_sb=None, hw2slots=None,
                    out2=g["out2"], g=g)


def _edge_layer(nc, tc, J, layer, table, row_w, nheads, hid, srcidx_sb,
                adidx_sb, adshard, b_bc, one_bc, ident, w2_sb, hw2slots,
                out2, g):
    NCH = _nchunk()
    NH = nheads
    DW = NH * hid                 # 256 or 40
    GW = row_w                    # 264 or 44
    ACOL = DW if layer == 1 else NCLS
    SCW = T1COL if layer == 1 else W2COL
    SCP = 512 if layer == 1 else 128
    sfx = f"l{layer}"
    EB = 2 if layer == 1 else 4
    colbase = [0]
    for j in J:
        colbase.append(colbase[-1] + j)

    with tc.tile_pool(name=f"g{sfx}", bufs=2) as gpool, \
         tc.tile_pool(name=f"m{sfx}", bufs=2) as mpool, \
         tc.tile_pool(name=f"e{sfx}", bufs=2) as epool, \
         tc.tile_pool(name=f"ac{sfx}", bufs=2, space="PSUM") as acp, \
         tc.tile_pool(name=f"tp{sfx}", bufs=1, space="PSUM") as tpp:
        acc = None
        eb0 = 0
        ebn = 0
        for c in range(NCH):
            Jc = J[c]
            G = gpool.tile([P, Jc, GW], bf16, tag="g")
            for t in range(Jc):
                cc = colbase[c] + t
                nc.gpsimd.indirect_dma_start(
                    out=G[:, t, :], out_offset=None, in_=table[:, :],
                    in_offset=bass.IndirectOffsetOnAxis(
                        ap=srcidx_sb[:, cc:cc + 1], axis=0))
            # alpha_dst for this chunk's 128 dsts
            adt = mpool.tile([P, NH], bf16, tag="adt")
            if layer == 1:
                nc.gpsimd.indirect_dma_start(
                    out=adt[:], out_offset=None, in_=adshard[:, :],
                    in_offset=bass.IndirectOffsetOnAxis(
                        ap=adidx_sb[:, c:c + 1], axis=0))
            else:
                nc.sync.dma_start(
                    out=adt[:],
                    in_=adshard[c * P:(c + 1) * P, NCLS + 1:NCLS + 2])

            # logit = alpha_src + alpha_dst; leaky_relu; exp (into G)
            lg = mpool.tile([P, Jc, NH], bf16, tag="lg")
            nc.vector.tensor_tensor(
                out=lg[:], in0=G[:, :, ACOL:ACOL + NH],
                in1=adt[:].unsqueeze(1).to_broadcast((P, Jc, NH)),
                op=OP.add)
            lr_n = mpool.tile([P, Jc, NH], bf16, tag="lrn")
            nc.vector.tensor_scalar(out=lr_n[:], in0=lg[:], scalar1=0.0,
                                    scalar2=NEG, op0=OP.min, op1=OP.mult)
            nc.vector.scalar_tensor_tensor(out=lg[:], in0=lg[:], scalar=0.0,
                                           in1=lr_n[:], op0=OP.max,
                                           op1=OP.add)
            nc.scalar.activation(out=G[:, :, ACOL:ACOL + NH], in_=lg[:],
                                 func=AF.Exp)
            if layer == 1:
                exe = mpool.tile([P, Jc, NH, hid], bf16, tag="exe")
                if c % 2 == 0:
                    nc.scalar.activation(
                        out=exe[:], in_=lg[:].unsqueeze(3)
                        .to_broadcast((P, Jc, NH, hid)), func=AF.Exp)
                else:
                    nc.vector.tensor_copy(
                        out=exe[:], in_=G[:, :, ACOL:ACOL + NH].unsqueeze(3)
                        .to_broadcast((P, Jc, NH, hid)))
                nc.vector.tensor_tensor(
                    out=G[:, :, :DW].rearrange("p t (h w) -> p t h w", h=NH),
                    in0=G[:, :, :DW].rearrange("p t (h w) -> p t h w", h=NH),
                    in1=exe[:], op=OP.mult)
            else:
                exe = mpool.tile([P, Jc, GW], bf16, tag="exe")
                nc.vector.tensor_copy(
                    out=exe[:], in_=G[:, :, ACOL:ACOL + 1]
                    .to_broadcast((P, Jc, GW)))
                nc.vector.tensor_tensor(out=G[:], in0=G[:], in1=exe[:],
                                        op=OP.mult)

            if c % EB == 0:
                eb0 = c
                ebn = min(EB, NCH - c)
                acc = acp.tile([P, ebn, SCP], f32, space="PSUM", tag="acc")
            ei = c - eb0
            for t in range(Jc):
                nc.tensor.matmul(out=acc[:, ei, :SCW], lhsT=ident[:],
                                 rhs=G[:, t, :SCW],
                                 start=(t == 0), stop=(t == Jc - 1))

            if ei != ebn - 1:
                continue

            # ---------------- epilogue for chunks eb0 .. eb0+ebn-1
            c0 = eb0
            EBv = ebn
            if layer == 2:
                den = epool.tile([P, EBv], f32, tag="den")
                nc.vector.tensor_scalar(out=den[:],
                                        in0=acc[:, :, NCLS + 2],
                                        scalar1=EPS, scalar2=None, op0=OP.max)
                rec = epool.tile([P, EBv], f32, tag="rec")
                nc.vector.reciprocal(out=rec[:], in_=den[:])
                o_sb = epool.tile([P, EBv, NCLS], f32, tag="osb")
                for e in range(EBv):
                    nc.vector.scalar_tensor_tensor(
                        out=o_sb[:, e, :], in0=acc[:, e, :NCLS],
                        scalar=rec[:, e:e + 1], in1=b_bc[:],
                        op0=OP.mult, op1=OP.add)
                    nc.sync.dma_start(
                        out=out2[(c0 + e) * P:(c0 + e + 1) * P, :],
                        in_=o_sb[:, e, :])
                continue

            den = epool.tile([P, EBv, NH], f32, tag="den")
            nc.vector.tensor_scalar(out=den[:], in0=acc[:, :, DW:DW + NH],
                                    scalar1=EPS, scalar2=None, op0=OP.max)
            rec = epool.tile([P, EBv, NH], f32, tag="rec")
            nc.vector.reciprocal(out=rec[:], in_=den[:])
            outv = epool.tile([P, EBv, DW], f32, tag="outv")
            nc.vector.tensor_tensor(
                out=outv[:].rearrange("p e (h w) -> p e h w", h=NH),
                in0=acc[:, :, :DW].rearrange("p e (h w) -> p e h w", h=NH),
                in1=rec[:].unsqueeze(3).to_broadcast((P, EBv, NH, hid)),
                op=OP.mult)
            nc.vector.tensor_tensor(
                out=outv[:], in0=outv[:],
                in1=b_bc[:].unsqueeze(1).to_broadcast((P, EBv, DW)),
                op=OP.add)
            mneg = epool.tile([P, EBv, DW], f32, tag="mneg")
            nc.vector.tensor_scalar(out=mneg[:], in0=outv[:], scalar1=0.0,
                                    scalar2=None, op0=OP.min)
            expm = epool.tile([P, EBv, DW], f32, tag="expm")
            nc.scalar.activation(out=expm[:], in_=mneg[:], func=AF.Exp)
            rel1 = epool.tile([P, EBv, DW], f32, tag="rel1")
            nc.vector.tensor_scalar(out=rel1[:], in0=outv[:], scalar1=0.0,
                                    scalar2=1.0, op0=OP.max, op1=OP.subtract)
            h_sb = epool.tile([P, EBv, DW], bf16, tag="h")
            nc.vector.tensor_tensor(out=h_sb[:], in0=expm[:], in1=rel1[:],
                                    op=OP.add)

            for e in range(EBv):
                hT_sb = epool.tile([P, 2, P], bf16, tag="hT")
                for hh in range(2):
                    hT_ps = tpp.tile([P, P], bf16, space="PSUM", tag="hT")
                    nc.tensor.transpose(out=hT_ps[:],
                                        in_=h_sb[:, e, hh * P:(hh + 1) * P],
                                        identity=ident[:])
                    nc.vector.tensor_copy(out=hT_sb[:, hh, :], in_=hT_ps[:])
                hw_ps = tpp.tile([P, W2COL], f32, space="PSUM", tag="hw")
                for hh in range(2):
                    nc.tensor.matmul(out=hw_ps[:], lhsT=hT_sb[:, hh, :],
                                     rhs=w2_sb[:, hh, :],
                                     start=(hh == 0), stop=(hh == 1))
                hw_sb = epool.tile([P, W2COL], bf16, tag="hws")
                nc.vector.tensor_tensor(out=hw_sb[:], in0=hw_ps[:],
                                        in1=one_bc[:], op=OP.add)
                r0 = (c0 + e) * P
                nc.sync.dma_start(out=hw2slots[r0:r0 + P, :], in_=hw_sb[:])
        if layer == 1:
            # sentinel row for table2
            with tc.tile_pool(name="s2", bufs=1) as s2p:
                sent_sb = s2p.tile([1, W2COL], bf16)
                nc.sync.dma_start(out=sent_sb[:], in_=g["sent2"][:, :])
                nc.sync.dma_start(out=hw2slots[NCH * P:NCH * P + 1, :],
                                  in_=sent_sb[:])


# ---------------------------------------------------------------- entry
def kernel(**inputs):
    global _last_bench
    args = {k: np.asarray(v) for k, v in inputs.items()}
    in_maps, J, perms = _prep(
        args["x"], args["edge_index"], args["W1"], args["a1_src"],
        args["a1_dst"], args["b1"], args["W2"], args["a2_src"],
        args["a2_dst"], args["b2"])
    if J not in _prog_cache:
        _prog_cache[J] = _build(J)
    nc = _prog_cache[J]
    trace = os.environ.get("GAT_TRACE", "0") == "1"
    r = bass_utils.run_bass_kernel_spmd(
        nc, in_maps, core_ids=list(range(NC)), trace=trace)
    _last_bench = r
    out = np.empty((N, NCLS), np.float32)
    for k in range(NC):
        o = r.results[k]["out2"]          # [NCH*P, 40] in rank order
        out[k * NSH:(k + 1) * NSH] = o[rankofinv(perms[k])]
    return out


def rankofinv(perm):
    """rows of out2 are rank-ordered; return index array mapping local dst
    id -> rank."""
    rank = np.empty(len(perm), np.int64)
    rank[perm] = np.arange(len(perm))
    return rank
